# revision 4
# baseline (speedup 1.0000x reference)
"""CommutatorConv2d kernel for Trainium2 (Bass/Tile), 8-core data-parallel.

Math: the reference's commutator/anticommutator conv reduces exactly to a
single-channel 3x3 conv on the channel-summed input:

    out[b] = T @ xs[b] @ A + Bm @ xs[b] @ T + bias,   xs = x.sum(axis=1)

where T is the 128x128 tridiagonal-ones matrix and A, Bm are tridiagonal
matrices built from K's column/row sums scaled by (lambda_c +/- lambda_a).

v3: everything bf16 (host casts once; harness gate is 2e-2, this lands
~4e-3): halves HBM traffic and runs the PE at 1 cycle/row. Layout
[H, B_loc, C, W] so a piece (channel slice) is one contiguous run per
partition; pieces are 16/12/4 channels -> 4KB/3KB/1KB DMA descriptors
(the HWDGE descriptor-generation rate, ~85M desc/s/queue, limits
throughput for small descriptors). Batch 1 + its stores ride the sync
ring; constants + batch 0 ride the scalar ring, so the two batches'
tails stagger naturally.

Fold: the PE folds each batch's leading 16 channels as 4 accumulating
N=512 identity matmuls (amortizes the PE's 173ns SBUF access latency);
ACT evacuates the [128,512] PSUM to bf16 mid-stream and the DVE finishes
it 512->128. The DVE (bf16 2x mode) trees the 12ch and 4ch pieces and
combines partials into xs. Sandwich per batch: uv = xs.T @ [T | BmT],
then out = uv1.T @ A + uv2.T @ T accumulated in PSUM, with ACT doing
PSUM evacuation and the fused bias add.
"""

import numpy as np

B, C, H, W = 16, 32, 128, 128
N_CORES = 8
B_LOC = B // N_CORES

PIECE_CH = (16, 12, 4)

_PROGRAM = None
LAST_RESULTS = None


def _build_program():
    import concourse.mybir as mybir
    from concourse import bacc
    from concourse.bass import MemorySpace
    from concourse.tile import TileContext

    bf16 = mybir.dt.bfloat16
    f32 = mybir.dt.float32
    nc = bacc.Bacc(
        "TRN2", target_bir_lowering=False, debug=False, num_devices=N_CORES
    )

    x_dram = nc.dram_tensor("x", (H, B_LOC, C, W), bf16, kind="ExternalInput")
    # fused constants: [A | T | BmT | I] as bf16 columns
    cm_dram = nc.dram_tensor("cmat", (H, 4 * W), bf16, kind="ExternalInput")
    bias_dram = nc.dram_tensor("biasv", (H, 1), f32, kind="ExternalInput")
    out_dram = nc.dram_tensor("out", (H, B_LOC, W), f32, kind="ExternalOutput")

    x_ap = x_dram.ap()
    out_ap = out_dram.ap()

    with TileContext(nc) as tc:
        with (
            tc.tile_pool(name="consts", bufs=1) as cpool,
            tc.tile_pool(name="xpool", bufs=1) as xpool,
            tc.tile_pool(name="spool", bufs=1) as spool,
            tc.tile_pool(name="psum", bufs=1, space=MemorySpace.PSUM) as ppool,
        ):
            cm_sb = cpool.tile([H, 4 * W], bf16, tag="cm")
            bias_sb = cpool.tile([H, 1], f32, tag="bias")
            a_sb = cm_sb[:, 0:W]
            t_sb = cm_sb[:, W : 2 * W]
            tbm_sb = cm_sb[:, W : 3 * W]  # [T | BmT]
            i_sb = cm_sb[:, 3 * W : 4 * W]

            # scalar ring: constants first, then batch 0's pieces.
            # sync ring: batch 1's pieces, then both stores (b1's first).
            nc.scalar.dma_start(out=cm_sb, in_=cm_dram.ap())
            nc.scalar.dma_start(out=bias_sb, in_=bias_dram.ap())
            tiles = {}
            for b, eng in ((1, nc.sync), (0, nc.scalar)):
                c0 = 0
                for p, nch in enumerate(PIECE_CH):
                    xq = xpool.tile([H, nch * W], bf16, tag=f"x{b}_{p}")
                    eng.dma_start(
                        out=xq.rearrange("h (c w) -> h c w", w=W),
                        in_=x_ap[:, b, c0 : c0 + nch, :],
                    )
                    tiles[(b, p)] = xq
                    c0 += nch

            # ---- PE fold of each batch's 16ch piece: 4 x N=512 quads ----
            fold_psum = {}
            for b in (1, 0):
                psum = ppool.tile([H, 4 * W], f32, tag=f"fold{b}")
                xq = tiles[(b, 0)]
                for q in range(4):
                    nc.tensor.matmul(
                        psum,
                        i_sb,
                        xq[:, q * 4 * W : (q + 1) * 4 * W],
                        start=(q == 0),
                        stop=(q == 3),
                        skip_group_check=True,
                    )
                fold_psum[b] = psum

            # ACT evacuates fold PSUMs to bf16 (mid-stream, hidden)
            p0_sb = {}
            for b in (1, 0):
                sb = spool.tile([H, 4 * W], bf16, tag=f"p0_{b}")
                nc.scalar.copy(sb, fold_psum[b])
                p0_sb[b] = sb

            # ---- DVE: finish 512->128, tree the 12ch + 4ch pieces ----
            def tree(ap_tile, nelem):
                # in-place halving tree down to W elems (nelem mult of 2W)
                n = nelem
                while n > W and n % 2 == 0 and (n // 2) % W == 0:
                    n //= 2
                    nc.vector.tensor_add(
                        ap_tile[:, :n], ap_tile[:, :n], ap_tile[:, n : 2 * n]
                    )
                # handle 3W leftover (12ch path: 1536->768->384 = 3W)
                if n == 3 * W:
                    nc.vector.tensor_add(
                        ap_tile[:, :W], ap_tile[:, :W], ap_tile[:, W : 2 * W]
                    )
                    nc.vector.tensor_add(
                        ap_tile[:, :W], ap_tile[:, :W], ap_tile[:, 2 * W : 3 * W]
                    )

            xs = {}
            for b in (1, 0):
                # finish the PE partial: 512 -> 128
                pp = p0_sb[b]
                tree(pp, 4 * W)
                # 12ch piece
                t12 = tiles[(b, 1)]
                tree(t12, 12 * W)
                # 4ch piece
                t4 = tiles[(b, 2)]
                tree(t4, 4 * W)
                # combine partials: xs = p0 + p12 + p4
                nc.vector.tensor_add(pp[:, :W], pp[:, :W], t12[:, :W])
                nc.vector.tensor_add(pp[:, :W], pp[:, :W], t4[:, :W])
                xs[b] = pp[:, :W]

            # ---- sandwich per batch (b1 first: its stream ends first) ----
            for b in (1, 0):
                uv_psum = ppool.tile([H, 2 * W], f32, tag=f"uv{b}p")
                nc.tensor.matmul(uv_psum, xs[b], tbm_sb, start=True, stop=True)
                uv_sb = spool.tile([H, 2 * W], bf16, tag=f"uv{b}")
                nc.scalar.copy(uv_sb, uv_psum)
                o_psum = ppool.tile([H, W], f32, tag=f"o{b}p")
                nc.tensor.matmul(
                    o_psum, uv_sb[:, 0:W], a_sb, start=True, stop=False,
                    skip_group_check=True,
                )
                nc.tensor.matmul(
                    o_psum, uv_sb[:, W : 2 * W], t_sb, start=False, stop=True,
                    skip_group_check=True,
                )
                o_sb = spool.tile([H, W], f32, tag=f"o{b}")
                nc.scalar.add(o_sb, o_psum, add=bias_sb)
                nc.sync.dma_start(out=out_ap[:, b, :], in_=o_sb)

    nc.compile()
    return nc


def _get_program():
    global _PROGRAM
    if _PROGRAM is None:
        _PROGRAM = _build_program()
    return _PROGRAM


def _build_consts(K, bias, lambda_c, lambda_a):
    import ml_dtypes

    K = np.asarray(K, np.float32)
    lc = float(np.asarray(lambda_c))
    la = float(np.asarray(lambda_a))
    a = (lc + la) * K.sum(axis=0)  # column sums -> horizontal taps
    b = (la - lc) * K.sum(axis=1)  # row sums -> vertical taps
    eye = np.eye(H, dtype=np.float32)
    up = np.eye(H, k=1, dtype=np.float32)
    dn = np.eye(H, k=-1, dtype=np.float32)
    T = eye + up + dn
    A = a[1] * eye + a[0] * up + a[2] * dn
    Bm = b[1] * eye + b[2] * up + b[0] * dn
    cm = np.concatenate([A, T, Bm.T, eye], axis=1)
    cm16 = np.ascontiguousarray(cm.astype(ml_dtypes.bfloat16))
    bias_col = np.full(
        (H, 1), np.asarray(bias, np.float32).reshape(-1)[0], np.float32
    )
    return cm16, bias_col


def kernel(x, K, bias, lambda_c, lambda_a, _trace=False):
    global LAST_RESULTS
    import ml_dtypes
    from concourse.bass_utils import run_bass_kernel_spmd

    x = np.asarray(x, np.float32)
    cm16, bias_col = _build_consts(K, bias, lambda_c, lambda_a)
    nc = _get_program()

    in_maps = []
    for core in range(N_CORES):
        shard = x[core * B_LOC : (core + 1) * B_LOC]  # [B_LOC, C, H, W]
        shard_t = np.ascontiguousarray(
            shard.transpose(2, 0, 1, 3).astype(ml_dtypes.bfloat16)
        )  # [H, B_LOC, C, W] bf16
        in_maps.append({"x": shard_t, "cmat": cm16, "biasv": bias_col})

    res = run_bass_kernel_spmd(
        nc, in_maps, core_ids=list(range(N_CORES)), trace=_trace
    )
    LAST_RESULTS = res
    out = np.concatenate(
        [r["out"].transpose(1, 0, 2) for r in res.results], axis=0
    )
    return out.reshape(B, 1, H, W).astype(np.float32, copy=False)
